# revision 1
# baseline (speedup 1.0000x reference)
"""DLRM pairwise-interaction kernel for Trainium2 (Bass/Tile), 8-core data parallel.

Problem: inputs [B=8192, N=64, D=128] fp32 ->
         out [B, 2016] fp32 = strictly-lower-tri (i-major) of per-sample Gram
         Z_b = X_b @ X_b^T.

Sharding: pure data parallel, B split into 8 shards of 1024 samples.

Per-core plan (1024 samples = 2 obatches of 512 = 8 blocks of 64 samples):
  - DMA in a 64-sample block as [128p, 4096]: partition = (pair-half, n),
    free = (pair c, d). Chunk c is the 2-sample stack [X_{2c}; X_{2c+1}].
  - TensorE transpose (fp32r) each chunk -> PSUM [128(d), 128(s,n)] = [Xa^T|Xb^T].
  - DVE copies 4 transposes at a time PSUM->SBUF (XT tile [128, 512]).
  - Gram matmul per pair (fp32r): lhsT = XT_c [128,128],
    rhs = 2-pair group [128,256] (N=256 hits the fast fp32r path; half the
    output is cross-sample garbage). Output slice is shifted by -(k%2)*128 so
    useful diag blocks land at k*256 + [0:64] (A) / +[64:128] (B) in PSUM.
  - ScalarE extracts diag blocks into Zbig [128, 256*64] for the obatch:
    Zbig[64*a + i, qq*64 + j] = Z_{2qq+a}[i, j], qq = pair index in obatch.
  - 63 out-DMAs per obatch: row index i moves [2 x 256 x i] strided
    (partition {i, 64+i}; sample stride uniform because consecutive pairs are
    adjacent samples) to out[s, T(i):T(i)+i], T(i)=i(i-1)/2.
"""

import numpy as np

import concourse.bass as bass
from concourse import bacc, tile, mybir
from concourse import bass_utils

F32 = mybir.dt.float32
F32R = mybir.dt.float32r

B_FULL = 8192
N_CORES = 8
B_CORE = B_FULL // N_CORES  # 1024
N = 64
D = 128
OUT_COLS = (N * (N - 1)) // 2  # 2016

BLK = 64                       # samples per input block
BLK_PAIRS = BLK // 2           # 32


def _tri(i: int) -> int:
    return (i * (i - 1)) // 2


def build_nc(b_core: int = B_CORE, repeats: int = 1, mode: str = "f32r",
             skip_out: bool = False, ob: int = 256, dma_cast: bool = True,
             out_only: bool = False, interleave: bool = True):
    """Build the Bass program for one core processing b_core samples.

    repeats > 1 wraps the whole workload in a hardware loop (timing only).
    mode: "f32r" (fp32r matmuls, ~1.6e-4 rel err on HW) or "bf16" (~2.5e-3).
    skip_out: replace packed-tril out-DMAs with one bulk dump (timing only).
    ob: samples per output batch (zbig size; 1024 -> 126 out-DMAs total).
    dma_cast: cast fp32->mm_dt inline in the input SWDGE DMA.
    """
    OB = ob
    OB_PAIRS = OB // 2
    BLKS_PER_OB = OB // BLK
    assert b_core % OB == 0
    n_ob = b_core // OB
    nc = bacc.Bacc("TRN2", target_bir_lowering=False, debug=False,
                   num_devices=N_CORES)
    x = nc.dram_tensor("x", [b_core, N, D], F32, kind="ExternalInput").ap()
    ident = nc.dram_tensor("ident", [128, 128], F32, kind="ExternalInput").ap()
    out = nc.dram_tensor("out", [b_core, OUT_COLS], F32,
                         kind="ExternalOutput").ap()

    bf16 = mode == "bf16"
    mm_dt = mybir.dt.bfloat16 if bf16 else F32R

    with tile.TileContext(nc) as tc:
        with (
            tc.tile_pool(name="xin", bufs=2) as xin_pool,
            tc.tile_pool(name="xbf", bufs=2) as xbf_pool,
            tc.tile_pool(name="xt", bufs=4) as xt_pool,
            tc.tile_pool(name="zbig", bufs=2 if ob <= 512 else 1) as zbig_pool,
            tc.tile_pool(name="const", bufs=1) as const_pool,
            tc.tile_pool(name="pst", bufs=2, space=bass.MemorySpace.PSUM) as pst_pool,
            tc.tile_pool(name="psz", bufs=2, space=bass.MemorySpace.PSUM) as psz_pool,
        ):
            ident_sb = const_pool.tile([128, 128], F32)
            nc.sync.dma_start(ident_sb[:], ident[:])
            # fp32r matmul operands must be explicitly rounded to fp32r
            # (BIR verifier enforces it), so both modes cast via tensor_copy.
            ident_mm = const_pool.tile([128, 128], mm_dt)
            nc.vector.tensor_copy(ident_mm[:], ident_sb[:])

            def emit_out_dmas(zbig, obi):
                """Return a list of thunks, one per out-DMA, for obatch obi."""
                outv = out[obi * OB:(obi + 1) * OB]
                outv = outv.rearrange("(q a) v -> a q v", a=2)
                thunks = []
                for i in range(1, N):
                    t0 = _tri(i)
                    for a in range(2):
                        def go(i=i, a=a, t0=t0, zbig=zbig, outv=outv):
                            srcz = zbig[64 * a + i: 64 * a + i + 1]
                            srcz = srcz.rearrange("p (q j) -> p q j", j=N)
                            # HWDGE is one TPB-level RTL block (sync/scalar is
                            # policy), so offload the widest rows to SWDGE --
                            # the only independent descriptor generator.
                            if i >= 48:
                                eng = nc.gpsimd
                            else:
                                eng = nc.sync if (i + a) % 2 == 0 else nc.scalar
                            eng.dma_start(
                                outv[a, :, t0:t0 + i].unsqueeze(0),
                                srcz[:, :, 0:i],
                            )
                        thunks.append(go)
                return thunks

            def body(_iv=None):
                pending = []
                for obi in range(n_ob):
                    zbig = zbig_pool.tile([128, OB_PAIRS * N], F32)
                    if out_only:
                        nc.gpsimd.memset(zbig[:], 0.0)
                        for th in emit_out_dmas(zbig, obi):
                            th()
                        continue
                    chunk = (len(pending) + BLKS_PER_OB - 1) // BLKS_PER_OB \
                        if pending else 0
                    for blk in range(BLKS_PER_OB):
                        s0 = obi * OB + blk * BLK
                        src = x[s0:s0 + BLK]
                        src = src.rearrange("(c two) n d -> (two n) c d", two=2)
                        if dma_cast:
                            # SWDGE casts fp32->mm_dt inline during the load
                            xsrc = xbf_pool.tile([128, BLK_PAIRS * D], mm_dt)
                            dst3 = xsrc[:].rearrange("p (c d) -> p c d",
                                                     c=BLK_PAIRS)
                            nc.gpsimd.dma_start(dst3, src)
                        else:
                            xin = xin_pool.tile([128, BLK_PAIRS * D], F32)
                            dst3 = xin[:].rearrange("p (c d) -> p c d",
                                                    c=BLK_PAIRS)
                            nc.gpsimd.dma_start(dst3, src)
                            xsrc = xbf_pool.tile([128, BLK_PAIRS * D], mm_dt)
                            nc.vector.tensor_copy(xsrc[:], xin[:])

                        for grp in range(BLK_PAIRS // 4):
                            pst = pst_pool.tile([128, 512], mm_dt)
                            xt = xt_pool.tile([128, 512], mm_dt)
                            for k in range(4):
                                c = grp * 4 + k
                                nc.tensor.transpose(
                                    pst[:, k * 128:(k + 1) * 128],
                                    xsrc[:, c * D:(c + 1) * D].bitcast(mm_dt),
                                    ident_mm[:].bitcast(mm_dt),
                                )
                            nc.vector.tensor_copy(xt[:], pst[:])

                            psz = psz_pool.tile([128, 1024], F32)
                            for k in range(4):
                                lhsT = xt[:, k * 128:(k + 1) * 128]
                                g2 = (k // 2) * 256
                                rhs = xt[:, g2:g2 + 256]
                                off = k * 256 - (k % 2) * 128
                                nc.tensor.matmul(
                                    psz[:, off:off + 256], lhsT, rhs,
                                    start=True, stop=True,
                                )
                            psz4 = psz[:].rearrange("p (k v) -> p k v", k=4)
                            qq0 = (blk * BLK_PAIRS + grp * 4) * N
                            dst = zbig[:, qq0:qq0 + 256]
                            dstA = dst[0:64].rearrange("p (k v) -> p k v", k=4)
                            dstB = dst[64:128].rearrange("p (k v) -> p k v", k=4)
                            nc.scalar.copy(dstA, psz4[0:64, :, 0:64])
                            nc.scalar.copy(dstB, psz4[64:128, :, 64:128])
                        # interleave previous obatch's out-DMAs between blocks
                        if pending:
                            for th in pending[:chunk]:
                                th()
                            pending = pending[chunk:]
                    if pending:
                        for th in pending:
                            th()
                        pending = []
                    if skip_out:
                        flat = zbig[:, 0:OUT_COLS * 2]
                        dstf = out[obi * OB:obi * OB + 256]
                        dstv = dstf.rearrange("(p r) v -> p (r v)", p=128)
                        nc.sync.dma_start(dstv, flat)
                        continue
                    if interleave and obi < n_ob - 1:
                        pending = emit_out_dmas(zbig, obi)
                    else:
                        for th in emit_out_dmas(zbig, obi):
                            th()

            if repeats == 1:
                body()
            else:
                with tc.For_i(0, repeats, 1) as _i:
                    body(_i)

    nc.compile()
    return nc


_CACHED = {"nc": None, "cfg": None}

# (mode, ob, dma_cast) in preference order; later entries are fallbacks in
# case a config fails compile/verification in the target environment.
_CONFIGS = [
    ("f32r", 256, True),
    ("f32r", 512, False),
    ("bf16", 256, True),
]


def kernel(inputs: np.ndarray) -> np.ndarray:
    """Full-input entry point: inputs [8192, 64, 128] fp32 -> [8192, 2016] fp32."""
    inputs = np.ascontiguousarray(np.asarray(inputs, dtype=np.float32))
    assert inputs.shape == (B_FULL, N, D), inputs.shape
    ident = np.eye(128, dtype=np.float32)
    in_maps = [
        {"x": inputs[c * B_CORE:(c + 1) * B_CORE], "ident": ident}
        for c in range(N_CORES)
    ]
    if _CACHED["nc"] is not None:
        res = bass_utils.run_bass_kernel_spmd(
            _CACHED["nc"], in_maps, core_ids=list(range(N_CORES)))
        return np.concatenate([r["out"] for r in res.results], axis=0)
    last_err = None
    for mode, ob, dc in _CONFIGS:
        try:
            nc = build_nc(mode=mode, ob=ob, dma_cast=dc)
            res = bass_utils.run_bass_kernel_spmd(
                nc, in_maps, core_ids=list(range(N_CORES)))
            _CACHED["nc"] = nc
            _CACHED["cfg"] = (mode, ob, dc)
            return np.concatenate([r["out"] for r in res.results], axis=0)
        except Exception as e:  # compile/verifier failure -> next config
            last_err = e
    raise last_err



# revision 10
# speedup vs baseline: 2.6403x; 2.6403x over previous
"""DLRM pairwise-interaction kernel for Trainium2 (Bass/Tile), 8-core data parallel.

Problem: inputs [B=8192, N=64, D=128] fp32 ->
         out [B, 2016] fp32 = strictly-lower-tri (i-major) of per-sample Gram
         Z_b = X_b @ X_b^T.

Sharding: pure data parallel, B split into 8 shards of 1024 samples.

v2 design (bf16 pipeline, contiguous out-DMA):
  Per core 1024 samples = 4 obatches (OB=256) x 4 blocks (BLK=64).
  1. SWDGE cast-load block [128p=(a,n), 32c x 128d] bf16 (a = sample in pair,
     c = pair chunk).
  2. PE transpose each chunk -> XT [128p=d, (c, a, n)] (stationary=data, FWL).
  3. Gram per pair c: matmul(lhsT=XT_c, rhs=XT_c) N=128 -> PSUM [128,128];
     diag blocks (p<64,f<64) = Z_A, (p>=64,f>=64) = Z_B.
  4. Extract diag blocks (cast fp32->bf16) into zbig [128p=(a,i), f=j*128+qq]
     (qq = pair index within obatch, 128 of them).
  5. Z-transpose per j (PE): zbig[:, j*128:(j+1)*128] -> ZT_j [128p=qq, (a,i)],
     copied into ztb [128p=qq, f = a*4096 + i*64 + j].  Now each partition
     holds both samples of its pair with j contiguous per (a, i).
  6. Pack tril rows: for i in 1..63 copy ztb[:, a, i*64 : i*64+i] ->
     zpk[:, a, T(i):T(i)+i] (cast bf16->fp32). Each partition now has the
     full 2016-entry packed row per sample.
  7. One HWDGE out-DMA per obatch: [128 qq, 2 a, 2016] -> out rows, 8064-byte
     contiguous HBM chunks (vs 504 sub-512B-chunk scatter DMAs in v1).
"""

import numpy as np

import concourse.bass as bass
from concourse import bacc, tile, mybir
from concourse import bass_utils

F32 = mybir.dt.float32
F32R = mybir.dt.float32r
BF16 = mybir.dt.bfloat16

B_FULL = 8192
N_CORES = 8
B_CORE = B_FULL // N_CORES  # 1024
N = 64
D = 128
OUT_COLS = (N * (N - 1)) // 2  # 2016

BLK = 64                       # samples per input block
BLK_PAIRS = BLK // 2           # 32
OB = 256                       # samples per obatch (Z-transpose granularity)
OB_PAIRS = OB // 2             # 128
BLKS_PER_OB = OB // BLK        # 4


def _tri(i: int) -> int:
    return (i * (i - 1)) // 2


def build_nc(b_core: int = B_CORE, repeats: int = 1, mode: str = "bf16",
             skip_out: bool = False):
    """Build the v2 Bass program for one core processing b_core samples.

    repeats > 1 wraps the workload in a hardware loop (timing only).
    mode: "bf16" (default) or "f32r" (Gram + transposes in fp32r; slower,
    tighter numerics).
    skip_out: drop Z-transpose/pack/out-DMA, bulk-dump zbig (timing only).
    """
    assert b_core % OB == 0
    n_ob = b_core // OB
    n_blk = b_core // BLK
    nc = bacc.Bacc("TRN2", target_bir_lowering=False, debug=False,
                   num_devices=N_CORES)
    x = nc.dram_tensor("x", [b_core, N, D], F32, kind="ExternalInput").ap()
    ident = nc.dram_tensor("ident", [128, 128], F32, kind="ExternalInput").ap()
    out = nc.dram_tensor("out", [b_core, OUT_COLS], F32,
                         kind="ExternalOutput").ap()

    mm_dt = BF16 if mode == "bf16" else F32R

    with tile.TileContext(nc) as tc:
        with (
            tc.tile_pool(name="xin", bufs=3) as xin_pool,
            tc.tile_pool(name="xt", bufs=2) as xt_pool,
            tc.tile_pool(name="zbig", bufs=2) as zbig_pool,
            tc.tile_pool(name="ztb", bufs=1) as ztb_pool,
            tc.tile_pool(name="zpk", bufs=2) as zpk_pool,
            tc.tile_pool(name="const", bufs=1) as const_pool,
            tc.tile_pool(name="pst", bufs=2, space=bass.MemorySpace.PSUM) as pst_pool,
            tc.tile_pool(name="psg", bufs=2, space=bass.MemorySpace.PSUM) as psg_pool,
            tc.tile_pool(name="psz", bufs=2, space=bass.MemorySpace.PSUM) as psz_pool,
        ):
            ident_sb = const_pool.tile([128, 128], F32)
            nc.sync.dma_start(ident_sb[:], ident[:])
            ident_mm = const_pool.tile([128, 128], mm_dt)
            nc.vector.tensor_copy(ident_mm[:], ident_sb[:])

            # per-obatch SBUF state, rotated via pools
            zbig_tiles = {}

            def emit_load_xt(gb):
                """Load block gb (cast fp32->mm_dt) and produce XT tile."""
                s0 = gb * BLK
                src = x[s0:s0 + BLK]
                src = src.rearrange("(c two) n d -> (two n) c d", two=2)
                xsrc = xin_pool.tile([128, BLK_PAIRS * D], mm_dt)
                dst3 = xsrc[:].rearrange("p (c d) -> p c d", c=BLK_PAIRS)
                nc.gpsimd.dma_start(dst3, src)

                xt_t = xt_pool.tile([128, BLK_PAIRS * D], mm_dt)
                for grp in range(BLK_PAIRS // 4):
                    pst = pst_pool.tile([128, 512], mm_dt)
                    for k in range(4):
                        c = grp * 4 + k
                        nc.tensor.transpose(
                            pst[:, k * 128:(k + 1) * 128],
                            xsrc[:, c * D:(c + 1) * D],
                            ident_mm[:],
                        )
                    nc.vector.tensor_copy(
                        xt_t[:, grp * 512:(grp + 1) * 512], pst[:])
                return xt_t

            def emit_gram(gb, xt_t):
                """Gram matmuls for block gb + extract diag blocks to zbig.

                Col-tiled: per pair c two M=64 matmuls share the PE array
                (col groups 0-1 for sample A, 2-3 for sample B via the
                auto-derived tile_position from out.base_partition), so
                Z_A rows land on partitions 0:64 and Z_B rows on 64:128 at
                the SAME free offset -> extract is one full-partition copy.
                """
                ob, blk = gb // BLKS_PER_OB, gb % BLKS_PER_OB
                if blk == 0:
                    zbig_tiles[ob] = zbig_pool.tile(
                        [128, 64 * OB_PAIRS], mm_dt, name=f"zbig{ob}")
                zbig_t = zbig_tiles[ob]
                zb3 = zbig_t[:].rearrange("p (j q) -> p j q", q=OB_PAIRS)
                GPT = 16  # pairs per psum tile ([128, 1024] = 2 banks)
                for half in range(BLK_PAIRS // GPT):
                    psg = psg_pool.tile([128, GPT * 64], F32)
                    for q in range(GPT):
                        c = half * GPT + q
                        lhsT_A = xt_t[:, c * D: c * D + 64]
                        lhsT_B = xt_t[:, c * D + 64: (c + 1) * D]
                        sl = slice(q * 64, (q + 1) * 64)
                        nc.tensor.matmul(psg[0:64, sl], lhsT_A, lhsT_A,
                                         start=True, stop=True)
                        nc.tensor.matmul(psg[64:128, sl], lhsT_B, lhsT_B,
                                         start=True, stop=True)
                    qq0 = blk * BLK_PAIRS + half * GPT
                    psv = psg[:].rearrange("p (q j) -> p j q", q=GPT)
                    nc.scalar.copy(zb3[:, :, qq0:qq0 + GPT], psv)

            def emit_zphase(ob):
                """Z-transpose + pack + out-DMA for a finished obatch."""
                zbig_t = zbig_tiles.pop(ob)
                ztb_t = ztb_pool.tile([128, 2 * N * N], mm_dt)
                zt3 = ztb_t[:].rearrange("p (ai j) -> p ai j", j=N)
                for jg in range(N // 4):
                    psz = psz_pool.tile([128, 512], mm_dt)
                    for k in range(4):
                        j = jg * 4 + k
                        nc.tensor.transpose(
                            psz[:, k * 128:(k + 1) * 128],
                            zbig_t[:, j * 128:(j + 1) * 128],
                            ident_mm[:],
                        )
                    psv = psz[:].rearrange("p (k v) -> p v k", k=4)
                    if jg % 2 == 0:
                        nc.scalar.copy(zt3[:, :, jg * 4:(jg + 1) * 4], psv)
                    else:
                        nc.vector.tensor_copy(zt3[:, :, jg * 4:(jg + 1) * 4],
                                              psv)
                if skip_out:
                    dstf = out[ob * OB:ob * OB + 128]
                    nc.gpsimd.dma_start(dstf[:, 0:2016], ztb_t[:, 0:2016])
                    return
                zpk_t = zpk_pool.tile([128, 2 * OUT_COLS], mm_dt)
                zpkv = zpk_t[:].rearrange("p (a t) -> p a t", a=2)
                ztv = ztb_t[:].rearrange("p (a f) -> p a f", a=2)
                for i in range(1, N):
                    t0 = _tri(i)
                    if i % 4 == 3:  # widest rows to gpsimd (SBUF->SBUF ok)
                        nc.gpsimd.tensor_copy(zpkv[:, :, t0:t0 + i],
                                              ztv[:, :, i * N:i * N + i])
                    else:
                        nc.vector.tensor_copy(zpkv[:, :, t0:t0 + i],
                                              ztv[:, :, i * N:i * N + i])
                outv = out[ob * OB:(ob + 1) * OB]
                outv = outv.rearrange("(q a) t -> q a t", a=2)
                # SWDGE casts bf16 -> fp32 inline; HBM chunks are 8064 B.
                nc.gpsimd.dma_start(outv, zpkv)

            def body(_iv=None):
                pending = None
                for gb in range(n_blk):
                    ob, blk = gb // BLKS_PER_OB, gb % BLKS_PER_OB
                    xt_t = emit_load_xt(gb)
                    if pending is not None:
                        emit_gram(*pending)
                    pending = (gb, xt_t)
                    if blk == 1 and ob > 0:
                        emit_zphase(ob - 1)
                emit_gram(*pending)
                emit_zphase(n_ob - 1)

            if repeats == 1:
                body()
            else:
                with tc.For_i(0, repeats, 1) as _i:
                    body(_i)

    nc.compile()
    return nc


# ---------------------------------------------------------------------------
# v1 (baseline) builder kept as fallback
# ---------------------------------------------------------------------------

def build_nc_v1(b_core: int = B_CORE, repeats: int = 1, mode: str = "f32r",
                skip_out: bool = False, ob: int = 256, dma_cast: bool = True,
                out_only: bool = False, interleave: bool = True):
    OB1 = ob
    OB_PAIRS1 = OB1 // 2
    BLKS_PER_OB1 = OB1 // BLK
    assert b_core % OB1 == 0
    n_ob = b_core // OB1
    nc = bacc.Bacc("TRN2", target_bir_lowering=False, debug=False,
                   num_devices=N_CORES)
    x = nc.dram_tensor("x", [b_core, N, D], F32, kind="ExternalInput").ap()
    ident = nc.dram_tensor("ident", [128, 128], F32, kind="ExternalInput").ap()
    out = nc.dram_tensor("out", [b_core, OUT_COLS], F32,
                         kind="ExternalOutput").ap()

    bf16 = mode == "bf16"
    mm_dt = mybir.dt.bfloat16 if bf16 else F32R

    with tile.TileContext(nc) as tc:
        with (
            tc.tile_pool(name="xin", bufs=2) as xin_pool,
            tc.tile_pool(name="xbf", bufs=2) as xbf_pool,
            tc.tile_pool(name="xt", bufs=4) as xt_pool,
            tc.tile_pool(name="zbig", bufs=2 if ob <= 512 else 1) as zbig_pool,
            tc.tile_pool(name="const", bufs=1) as const_pool,
            tc.tile_pool(name="pst", bufs=2, space=bass.MemorySpace.PSUM) as pst_pool,
            tc.tile_pool(name="psz", bufs=2, space=bass.MemorySpace.PSUM) as psz_pool,
        ):
            ident_sb = const_pool.tile([128, 128], F32)
            nc.sync.dma_start(ident_sb[:], ident[:])
            ident_mm = const_pool.tile([128, 128], mm_dt)
            nc.vector.tensor_copy(ident_mm[:], ident_sb[:])

            def emit_out_dmas(zbig, obi):
                outv = out[obi * OB1:(obi + 1) * OB1]
                outv = outv.rearrange("(q a) v -> a q v", a=2)
                thunks = []
                for i in range(1, N):
                    t0 = _tri(i)
                    for a in range(2):
                        def go(i=i, a=a, t0=t0, zbig=zbig, outv=outv):
                            srcz = zbig[64 * a + i: 64 * a + i + 1]
                            srcz = srcz.rearrange("p (q j) -> p q j", j=N)
                            if i >= 48:
                                eng = nc.gpsimd
                            else:
                                eng = nc.sync if (i + a) % 2 == 0 else nc.scalar
                            eng.dma_start(
                                outv[a, :, t0:t0 + i].unsqueeze(0),
                                srcz[:, :, 0:i],
                            )
                        thunks.append(go)
                return thunks

            def body(_iv=None):
                pending = []
                for obi in range(n_ob):
                    zbig = zbig_pool.tile([128, OB_PAIRS1 * N], F32)
                    chunk = (len(pending) + BLKS_PER_OB1 - 1) // BLKS_PER_OB1 \
                        if pending else 0
                    for blk in range(BLKS_PER_OB1):
                        s0 = obi * OB1 + blk * BLK
                        src = x[s0:s0 + BLK]
                        src = src.rearrange("(c two) n d -> (two n) c d", two=2)
                        xsrc = xbf_pool.tile([128, BLK_PAIRS * D], mm_dt)
                        dst3 = xsrc[:].rearrange("p (c d) -> p c d",
                                                 c=BLK_PAIRS)
                        nc.gpsimd.dma_start(dst3, src)

                        for grp in range(BLK_PAIRS // 4):
                            pst = pst_pool.tile([128, 512], mm_dt)
                            xt = xt_pool.tile([128, 512], mm_dt)
                            for k in range(4):
                                c = grp * 4 + k
                                nc.tensor.transpose(
                                    pst[:, k * 128:(k + 1) * 128],
                                    xsrc[:, c * D:(c + 1) * D].bitcast(mm_dt),
                                    ident_mm[:].bitcast(mm_dt),
                                )
                            nc.vector.tensor_copy(xt[:], pst[:])

                            psz = psz_pool.tile([128, 1024], F32)
                            for k in range(4):
                                lhsT = xt[:, k * 128:(k + 1) * 128]
                                g2 = (k // 2) * 256
                                rhs = xt[:, g2:g2 + 256]
                                off = k * 256 - (k % 2) * 128
                                nc.tensor.matmul(
                                    psz[:, off:off + 256], lhsT, rhs,
                                    start=True, stop=True,
                                )
                            psz4 = psz[:].rearrange("p (k v) -> p k v", k=4)
                            qq0 = (blk * BLK_PAIRS + grp * 4) * N
                            dst = zbig[:, qq0:qq0 + 256]
                            dstA = dst[0:64].rearrange("p (k v) -> p k v", k=4)
                            dstB = dst[64:128].rearrange("p (k v) -> p k v", k=4)
                            nc.scalar.copy(dstA, psz4[0:64, :, 0:64])
                            nc.scalar.copy(dstB, psz4[64:128, :, 64:128])
                        if pending:
                            for th in pending[:chunk]:
                                th()
                            pending = pending[chunk:]
                    if pending:
                        for th in pending:
                            th()
                        pending = []
                    if interleave and obi < n_ob - 1:
                        pending = emit_out_dmas(zbig, obi)
                    else:
                        for th in emit_out_dmas(zbig, obi):
                            th()

            if repeats == 1:
                body()
            else:
                with tc.For_i(0, repeats, 1) as _i:
                    body(_i)

    nc.compile()
    return nc


_CACHED = {"nc": None, "cfg": None}

# (builder, mode) in preference order; later entries are fallbacks in case a
# config fails compile/verification in the target environment.
_CONFIGS = [
    ("v2", "bf16"),
    ("v2", "f32r"),
    ("v1", "f32r"),
]


def kernel(inputs: np.ndarray) -> np.ndarray:
    """Full-input entry point: inputs [8192, 64, 128] fp32 -> [8192, 2016] fp32."""
    inputs = np.ascontiguousarray(np.asarray(inputs, dtype=np.float32))
    assert inputs.shape == (B_FULL, N, D), inputs.shape
    ident = np.eye(128, dtype=np.float32)
    in_maps = [
        {"x": inputs[c * B_CORE:(c + 1) * B_CORE], "ident": ident}
        for c in range(N_CORES)
    ]
    if _CACHED["nc"] is not None:
        res = bass_utils.run_bass_kernel_spmd(
            _CACHED["nc"], in_maps, core_ids=list(range(N_CORES)))
        return np.concatenate([r["out"] for r in res.results], axis=0)
    last_err = None
    for builder, mode in _CONFIGS:
        try:
            if builder == "v2":
                nc = build_nc(mode=mode)
            else:
                nc = build_nc_v1(mode=mode)
            res = bass_utils.run_bass_kernel_spmd(
                nc, in_maps, core_ids=list(range(N_CORES)))
            _CACHED["nc"] = nc
            _CACHED["cfg"] = (builder, mode)
            return np.concatenate([r["out"] for r in res.results], axis=0)
        except Exception as e:  # compile/verifier failure -> next config
            last_err = e
    raise last_err


# revision 25
# speedup vs baseline: 3.2234x; 1.2209x over previous
"""DLRM pairwise-interaction kernel for Trainium2 (Bass/Tile), 8-core data parallel.

Problem: inputs [B=8192, N=64, D=128] fp32 ->
         out [B, 2016] fp32 = strictly-lower-tri (i-major) of per-sample Gram
         Z_b = X_b @ X_b^T.

Sharding: pure data parallel, B split into 8 shards of 1024 samples.

v2 design (bf16 pipeline, contiguous out-DMA):
  Per core 1024 samples = 4 obatches (OB=256) x 4 blocks (BLK=64).
  1. SWDGE cast-load block [128p=(a,n), 32c x 128d] bf16 (a = sample in pair,
     c = pair chunk).
  2. PE transpose each chunk -> XT [128p=d, (c, a, n)] (stationary=data, FWL).
  3. Gram per pair c: matmul(lhsT=XT_c, rhs=XT_c) N=128 -> PSUM [128,128];
     diag blocks (p<64,f<64) = Z_A, (p>=64,f>=64) = Z_B.
  4. Extract diag blocks (cast fp32->bf16) into zbig [128p=(a,i), f=j*128+qq]
     (qq = pair index within obatch, 128 of them).
  5. Z-transpose per j (PE): zbig[:, j*128:(j+1)*128] -> ZT_j [128p=qq, (a,i)],
     copied into ztb [128p=qq, f = a*4096 + i*64 + j].  Now each partition
     holds both samples of its pair with j contiguous per (a, i).
  6. Pack tril rows: for i in 1..63 copy ztb[:, a, i*64 : i*64+i] ->
     zpk[:, a, T(i):T(i)+i] (cast bf16->fp32). Each partition now has the
     full 2016-entry packed row per sample.
  7. One HWDGE out-DMA per obatch: [128 qq, 2 a, 2016] -> out rows, 8064-byte
     contiguous HBM chunks (vs 504 sub-512B-chunk scatter DMAs in v1).
"""

import numpy as np

import concourse.bass as bass
from concourse import bacc, tile, mybir
from concourse import bass_utils

F32 = mybir.dt.float32
F32R = mybir.dt.float32r
BF16 = mybir.dt.bfloat16

B_FULL = 8192
N_CORES = 8
B_CORE = B_FULL // N_CORES  # 1024
N = 64
D = 128
OUT_COLS = (N * (N - 1)) // 2  # 2016

BLK = 64                       # samples per input block
BLK_PAIRS = BLK // 2           # 32
OB = 256                       # samples per obatch (Z-transpose granularity)
OB_PAIRS = OB // 2             # 128
BLKS_PER_OB = OB // BLK        # 4


def _tri(i: int) -> int:
    return (i * (i - 1)) // 2


def build_nc(b_core: int = B_CORE, repeats: int = 1, mode: str = "bf16",
             skip_out: bool = False, stage: str = "full", gpt: int = 16,
             pstb: int = 4, pszb: int = 0, rr: int = 0, spread: bool = False):
    """Build the v2 Bass program for one core processing b_core samples.

    repeats > 1 wraps the workload in a hardware loop (timing only).
    mode: "bf16" (default) or "f32r" (Gram + transposes in fp32r; slower,
    tighter numerics).
    skip_out: drop pack/out-DMA, bulk-dump ztb (timing only).
    stage: truncate the pipeline for timing attribution (timing only):
      "load" (in-DMA only), "xt" (+PE transpose+copy), "gram" (+Gram+extract),
      "zt" (+Z-transpose+copy), "full".
    """
    STAGES = ["load", "xt", "gram", "zt", "full"]
    stage_i = STAGES.index(stage)
    assert b_core % OB == 0
    n_ob = b_core // OB
    n_blk = b_core // BLK
    nc = bacc.Bacc("TRN2", target_bir_lowering=False, debug=False,
                   num_devices=N_CORES)
    x = nc.dram_tensor("x", [b_core, N, D], F32, kind="ExternalInput").ap()
    ident = nc.dram_tensor("ident", [128, 128], F32, kind="ExternalInput").ap()
    out = nc.dram_tensor("out", [b_core, OUT_COLS], F32,
                         kind="ExternalOutput").ap()

    mm_dt = BF16 if mode == "bf16" else F32R

    with tile.TileContext(nc) as tc:
        with (
            tc.tile_pool(name="xin", bufs=3) as xin_pool,
            tc.tile_pool(name="xt", bufs=2) as xt_pool,
            tc.tile_pool(name="zbig", bufs=2) as zbig_pool,
            tc.tile_pool(name="ztb", bufs=1) as ztb_pool,
            tc.tile_pool(name="zpk", bufs=2) as zpk_pool,
            tc.tile_pool(name="const", bufs=1) as const_pool,
            tc.tile_pool(name="pst", bufs=pstb, space=bass.MemorySpace.PSUM) as pst_pool,
            tc.tile_pool(name="psg", bufs=2, space=bass.MemorySpace.PSUM) as psg_pool,
        ):
            ident_sb = const_pool.tile([128, 128], F32)
            nc.sync.dma_start(ident_sb[:], ident[:])
            ident_mm = const_pool.tile([128, 128], mm_dt)
            nc.vector.tensor_copy(ident_mm[:], ident_sb[:])

            # per-obatch SBUF state, rotated via pools
            zbig_tiles = {}
            zstate = {}

            # All PSUM->SBUF drains round-robin ACT:DVE at 4:3 (ACT is
            # slightly faster per drain; DVE also owns the pack copies).
            drain_ct = [0]

            def drain(dst, src):
                k = drain_ct[0]
                drain_ct[0] += 1
                if (k % 7 < 4) if rr else (k % 2 == 0):
                    nc.scalar.copy(dst, src)
                else:
                    nc.vector.tensor_copy(dst, src)

            def emit_load_xt(gb):
                """Load block gb (cast fp32->mm_dt) and produce XT tile."""
                s0 = gb * BLK
                src = x[s0:s0 + BLK]
                src = src.rearrange("(c two) n d -> (two n) c d", two=2)
                xsrc = xin_pool.tile([128, BLK_PAIRS * D], mm_dt)
                dst3 = xsrc[:].rearrange("p (c d) -> p c d", c=BLK_PAIRS)
                nc.gpsimd.dma_start(dst3, src)
                if stage_i < 1:
                    return None

                # Transpose via NORMAL-mode matmul: out = lhsT.T @ I = chunk.T.
                # Unlike is_transpose, this hits the FWL weight-load path and
                # counts as PE-busy for HAM (~81 ns/tile vs ~214 ns), at the
                # price of an fp32 PSUM drain.
                xt_t = xt_pool.tile([128, BLK_PAIRS * D], mm_dt)
                for grp in range(BLK_PAIRS // 4):
                    pst = pst_pool.tile([128, 512], F32, name="pst")
                    for k in range(4):
                        c = grp * 4 + k
                        nc.tensor.matmul(
                            pst[:, k * 128:(k + 1) * 128],
                            xsrc[:, c * D:(c + 1) * D],
                            ident_mm[:],
                            start=True, stop=True,
                        )
                    drain(xt_t[:, grp * 512:(grp + 1) * 512], pst[:])
                return xt_t

            def emit_gram(gb, xt_t):
                """Gram matmuls for block gb + extract diag blocks to zbig.

                Col-tiled: per pair c two M=64 matmuls share the PE array
                (col groups 0-1 for sample A, 2-3 for sample B via the
                auto-derived tile_position from out.base_partition), so
                Z_A rows land on partitions 0:64 and Z_B rows on 64:128 at
                the SAME free offset -> extract is one full-partition copy.
                """
                if stage_i < 2:
                    return
                ob, blk = gb // BLKS_PER_OB, gb % BLKS_PER_OB
                if blk == 0:
                    zbig_tiles[ob] = zbig_pool.tile(
                        [128, 64 * OB_PAIRS], mm_dt, name="zbig_t")
                zbig_t = zbig_tiles[ob]
                zb3 = zbig_t[:].rearrange("p (j q) -> p j q", q=OB_PAIRS)
                GPT = gpt  # pairs per psum tile
                for half in range(BLK_PAIRS // GPT):
                    psg = psg_pool.tile([128, GPT * 64], F32)
                    for q in range(GPT):
                        c = half * GPT + q
                        lhsT_A = xt_t[:, c * D: c * D + 64]
                        lhsT_B = xt_t[:, c * D + 64: (c + 1) * D]
                        sl = slice(q * 64, (q + 1) * 64)
                        nc.tensor.matmul(psg[0:64, sl], lhsT_A, lhsT_A,
                                         start=True, stop=True)
                        nc.tensor.matmul(psg[64:128, sl], lhsT_B, lhsT_B,
                                         start=True, stop=True)
                    qq0 = blk * BLK_PAIRS + half * GPT
                    psv = psg[:].rearrange("p (q j) -> p j q", q=GPT)
                    drain(zb3[:, :, qq0:qq0 + GPT], psv)

            def emit_zchunk(ob, part):
                """One quarter of an obatch's Z-phase: 16 Z-transposes
                (j in [16*part, 16*part+16)), their drains, the pack copies
                for the rows those j's complete, and a split out-DMA.
                Spreading this across the next obatch's blocks keeps every
                engine's per-block work even."""
                if stage_i < 3:
                    return
                if part == 0:
                    zb = zbig_tiles.pop(ob)
                    ztb_t = ztb_pool.tile([128, 2 * N * N], mm_dt,
                                          name="ztb_t")
                    zpk_t = zpk_pool.tile([128, 2 * OUT_COLS], mm_dt,
                                          name="zpk_t")
                    zstate[ob] = (zb, ztb_t, zpk_t)
                zbig_t, ztb_t, zpk_t = zstate[ob]
                zt3 = ztb_t[:].rearrange("p (ai j) -> p ai j", j=N)
                for jg in range(part * 4, part * 4 + 4):
                    psz = pst_pool.tile([128, 512], F32, name="pst")
                    for k in range(4):
                        j = jg * 4 + k
                        nc.tensor.matmul(
                            psz[:, k * 128:(k + 1) * 128],
                            zbig_t[:, j * 128:(j + 1) * 128],
                            ident_mm[:],
                            start=True, stop=True,
                        )
                    psv = psz[:].rearrange("p (k v) -> p v k", k=4)
                    drain(zt3[:, :, jg * 4:(jg + 1) * 4], psv)
                if skip_out or stage_i < 4:
                    if part == 3:
                        dstf = out[ob * OB:ob * OB + 128]
                        nc.gpsimd.dma_start(dstf[:, 0:2016], ztb_t[:, 0:2016])
                        zstate.pop(ob)
                    return
                zpkv = zpk_t[:].rearrange("p (a t) -> p a t", a=2)
                ztv = ztb_t[:].rearrange("p (a f) -> p a f", a=2)
                # rows i in (16*part, 16*part+16] only need j < 16*(part+1)
                for i in range(16 * part + 1, min(N, 16 * part + 17)):
                    t0 = _tri(i)
                    nc.vector.tensor_copy(zpkv[:, :, t0:t0 + i],
                                          ztv[:, :, i * N:i * N + i])
                outv = out[ob * OB:(ob + 1) * OB]
                outv = outv.rearrange("(q a) t -> q a t", a=2)
                T33 = _tri(33)  # rows 1..32 are packed after part 1
                # SWDGE casts bf16 -> fp32 inline; HBM chunks >= 2112 B.
                if part == 1:
                    nc.gpsimd.dma_start(outv[:, :, 0:T33], zpkv[:, :, 0:T33])
                elif part == 3:
                    nc.gpsimd.dma_start(outv[:, :, T33:OUT_COLS],
                                        zpkv[:, :, T33:OUT_COLS])
                    zstate.pop(ob)

            def body(_iv=None):
                pending = None
                for gb in range(n_blk):
                    ob, blk = gb // BLKS_PER_OB, gb % BLKS_PER_OB
                    xt_t = emit_load_xt(gb)
                    if pending is not None:
                        emit_gram(*pending)
                    pending = (gb, xt_t)
                    if ob > 0:
                        if spread:
                            emit_zchunk(ob - 1, blk)
                        elif blk == 1:
                            for part in range(4):
                                emit_zchunk(ob - 1, part)
                emit_gram(*pending)
                for part in range(4):
                    emit_zchunk(n_ob - 1, part)

            if repeats == 1:
                body()
            else:
                with tc.For_i(0, repeats, 1) as _i:
                    body(_i)

    nc.compile()
    return nc


# ---------------------------------------------------------------------------
# v1 (baseline) builder kept as fallback
# ---------------------------------------------------------------------------

def build_nc_v1(b_core: int = B_CORE, repeats: int = 1, mode: str = "f32r",
                skip_out: bool = False, ob: int = 256, dma_cast: bool = True,
                out_only: bool = False, interleave: bool = True):
    OB1 = ob
    OB_PAIRS1 = OB1 // 2
    BLKS_PER_OB1 = OB1 // BLK
    assert b_core % OB1 == 0
    n_ob = b_core // OB1
    nc = bacc.Bacc("TRN2", target_bir_lowering=False, debug=False,
                   num_devices=N_CORES)
    x = nc.dram_tensor("x", [b_core, N, D], F32, kind="ExternalInput").ap()
    ident = nc.dram_tensor("ident", [128, 128], F32, kind="ExternalInput").ap()
    out = nc.dram_tensor("out", [b_core, OUT_COLS], F32,
                         kind="ExternalOutput").ap()

    bf16 = mode == "bf16"
    mm_dt = mybir.dt.bfloat16 if bf16 else F32R

    with tile.TileContext(nc) as tc:
        with (
            tc.tile_pool(name="xin", bufs=2) as xin_pool,
            tc.tile_pool(name="xbf", bufs=2) as xbf_pool,
            tc.tile_pool(name="xt", bufs=4) as xt_pool,
            tc.tile_pool(name="zbig", bufs=2 if ob <= 512 else 1) as zbig_pool,
            tc.tile_pool(name="const", bufs=1) as const_pool,
            tc.tile_pool(name="pst", bufs=2, space=bass.MemorySpace.PSUM) as pst_pool,
            tc.tile_pool(name="psz", bufs=2, space=bass.MemorySpace.PSUM) as psz_pool,
        ):
            ident_sb = const_pool.tile([128, 128], F32)
            nc.sync.dma_start(ident_sb[:], ident[:])
            ident_mm = const_pool.tile([128, 128], mm_dt)
            nc.vector.tensor_copy(ident_mm[:], ident_sb[:])

            def emit_out_dmas(zbig, obi):
                outv = out[obi * OB1:(obi + 1) * OB1]
                outv = outv.rearrange("(q a) v -> a q v", a=2)
                thunks = []
                for i in range(1, N):
                    t0 = _tri(i)
                    for a in range(2):
                        def go(i=i, a=a, t0=t0, zbig=zbig, outv=outv):
                            srcz = zbig[64 * a + i: 64 * a + i + 1]
                            srcz = srcz.rearrange("p (q j) -> p q j", j=N)
                            if i >= 48:
                                eng = nc.gpsimd
                            else:
                                eng = nc.sync if (i + a) % 2 == 0 else nc.scalar
                            eng.dma_start(
                                outv[a, :, t0:t0 + i].unsqueeze(0),
                                srcz[:, :, 0:i],
                            )
                        thunks.append(go)
                return thunks

            def body(_iv=None):
                pending = []
                for obi in range(n_ob):
                    zbig = zbig_pool.tile([128, OB_PAIRS1 * N], F32)
                    chunk = (len(pending) + BLKS_PER_OB1 - 1) // BLKS_PER_OB1 \
                        if pending else 0
                    for blk in range(BLKS_PER_OB1):
                        s0 = obi * OB1 + blk * BLK
                        src = x[s0:s0 + BLK]
                        src = src.rearrange("(c two) n d -> (two n) c d", two=2)
                        xsrc = xbf_pool.tile([128, BLK_PAIRS * D], mm_dt)
                        dst3 = xsrc[:].rearrange("p (c d) -> p c d",
                                                 c=BLK_PAIRS)
                        nc.gpsimd.dma_start(dst3, src)

                        for grp in range(BLK_PAIRS // 4):
                            pst = pst_pool.tile([128, 512], mm_dt)
                            xt = xt_pool.tile([128, 512], mm_dt)
                            for k in range(4):
                                c = grp * 4 + k
                                nc.tensor.transpose(
                                    pst[:, k * 128:(k + 1) * 128],
                                    xsrc[:, c * D:(c + 1) * D].bitcast(mm_dt),
                                    ident_mm[:].bitcast(mm_dt),
                                )
                            nc.vector.tensor_copy(xt[:], pst[:])

                            psz = psz_pool.tile([128, 1024], F32)
                            for k in range(4):
                                lhsT = xt[:, k * 128:(k + 1) * 128]
                                g2 = (k // 2) * 256
                                rhs = xt[:, g2:g2 + 256]
                                off = k * 256 - (k % 2) * 128
                                nc.tensor.matmul(
                                    psz[:, off:off + 256], lhsT, rhs,
                                    start=True, stop=True,
                                )
                            psz4 = psz[:].rearrange("p (k v) -> p k v", k=4)
                            qq0 = (blk * BLK_PAIRS + grp * 4) * N
                            dst = zbig[:, qq0:qq0 + 256]
                            dstA = dst[0:64].rearrange("p (k v) -> p k v", k=4)
                            dstB = dst[64:128].rearrange("p (k v) -> p k v", k=4)
                            nc.scalar.copy(dstA, psz4[0:64, :, 0:64])
                            nc.scalar.copy(dstB, psz4[64:128, :, 64:128])
                        if pending:
                            for th in pending[:chunk]:
                                th()
                            pending = pending[chunk:]
                    if pending:
                        for th in pending:
                            th()
                        pending = []
                    if interleave and obi < n_ob - 1:
                        pending = emit_out_dmas(zbig, obi)
                    else:
                        for th in emit_out_dmas(zbig, obi):
                            th()

            if repeats == 1:
                body()
            else:
                with tc.For_i(0, repeats, 1) as _i:
                    body(_i)

    nc.compile()
    return nc


_CACHED = {"nc": None, "cfg": None}

# (builder, mode) in preference order; later entries are fallbacks in case a
# config fails compile/verification in the target environment.
_CONFIGS = [
    ("v2", "bf16"),
    ("v2", "f32r"),
    ("v1", "f32r"),
]


def kernel(inputs: np.ndarray) -> np.ndarray:
    """Full-input entry point: inputs [8192, 64, 128] fp32 -> [8192, 2016] fp32."""
    inputs = np.ascontiguousarray(np.asarray(inputs, dtype=np.float32))
    assert inputs.shape == (B_FULL, N, D), inputs.shape
    ident = np.eye(128, dtype=np.float32)
    in_maps = [
        {"x": inputs[c * B_CORE:(c + 1) * B_CORE], "ident": ident}
        for c in range(N_CORES)
    ]
    if _CACHED["nc"] is not None:
        res = bass_utils.run_bass_kernel_spmd(
            _CACHED["nc"], in_maps, core_ids=list(range(N_CORES)))
        return np.concatenate([r["out"] for r in res.results], axis=0)
    last_err = None
    for builder, mode in _CONFIGS:
        try:
            if builder == "v2":
                nc = build_nc(mode=mode)
            else:
                nc = build_nc_v1(mode=mode)
            res = bass_utils.run_bass_kernel_spmd(
                nc, in_maps, core_ids=list(range(N_CORES)))
            _CACHED["nc"] = nc
            _CACHED["cfg"] = (builder, mode)
            return np.concatenate([r["out"] for r in res.results], axis=0)
        except Exception as e:  # compile/verifier failure -> next config
            last_err = e
    raise last_err
